# revision 7
# baseline (speedup 1.0000x reference)
"""Trainium2 Bass kernel for nn_Interval_Refine (ragged bidirectional GRU +
quantized MLP interval refinement).

Sharding: 6 of 8 cores each own one (scale, direction) GRU group — the
sequential scan is the bottleneck, and groups are independent. Host packs
each group's member-node sequences (sorted by length, rectangular I=40
columns) ; frozen steps (t >= eff) get +50 on the z-gate pre-activation so
z == 1 exactly in fp32 and h carries unchanged. Per scan step the core does
12 accumulating matmuls (h @ whh.T in transposed/folded layout) plus fused
gate math on ACT/DVE; the input-side gi = x @ wih.T matmuls are interleaved
one-per-step into the PE stream so they run inside the gate-latency gaps.
After the scan an AllGather shares final hiddens; every core redundantly
computes the (int8-fake-quant) MLP head + soft-argmax boundary refinement.
"""

import math

import numpy as np

COUNTS = [15, 40, 15]
OFFS = [0, 15, 55, 70]
BINS = [80, 60, 80]
MAX_T = 512
IPAD = 40
D = 256
H = 256
S = 12  # scan steps per gi chunk
W3PAD = 168
# core -> (scale, dir); cores 6,7 duplicate scale 1 (their slots are ignored)
CORE_GROUPS = [(0, 0), (0, 1), (1, 0), (1, 1), (2, 0), (2, 1), (1, 0), (1, 1)]

_prog_cache = {}
PROFILE = False          # set True to capture an NTFF profile on the next run
TRACE_CORES = None       # e.g. [4] to profile the scale-2 fwd core
LAST_RESULT = None       # BassKernelResults of the most recent run


def _quant(w):
    w = np.asarray(w, np.float32)
    s = np.maximum((np.abs(w).max() / np.float32(127.0)).astype(np.float32),
                   np.float32(1e-8))
    q = np.clip(np.round(w / s), -128, 127).astype(np.float32) * s
    return q.astype(np.float32)


def _softmax(x, axis=-1):
    m = x.max(axis=axis, keepdims=True)
    e = np.exp(x - m)
    return e / e.sum(axis=axis, keepdims=True)


def _preprocess(inputs):
    ne = np.asarray(inputs["node_embeddings"], np.float32)
    tp = np.asarray(inputs["time_positions"], np.float32)[:, 0]
    al = np.float32(np.asarray(inputs["audio_len"], np.float32)[0])
    anc = np.asarray(inputs["anchor_intervals"], np.float32)
    tpos = tp * al

    members, counts = [], []
    for i in range(70):
        m = np.nonzero((tpos >= anc[i, 0]) & (tpos <= anc[i, 1]))[0][:MAX_T]
        members.append(m)
        counts.append(len(m))
    counts = np.array(counts)
    eff = np.clip(counts, 1, MAX_T)
    orders = [OFFS[s] + np.argsort(-eff[OFFS[s]:OFFS[s + 1]], kind="stable")
              for s in range(3)]
    T = int(eff.max())
    abn = _softmax(np.asarray(inputs["node_pred"], np.float32), axis=-1)[:, 0]
    return dict(ne=ne, members=members, counts=counts, eff=eff, orders=orders,
                T=T, tpos=tpos, al=al, anc=anc, abn=abn)


def _build_program(T_pad):
    import concourse.bacc as bacc
    import concourse.mybir as mybir
    from concourse.tile import TileContext

    f32 = mybir.dt.float32
    Alu = mybir.AluOpType
    Act = mybir.ActivationFunctionType

    NCOLS = IPAD * T_pad
    CH = IPAD * S  # 480 columns per gi chunk
    n_chunks = T_pad // S

    nc = bacc.Bacc("TRN2", target_bir_lowering=False, debug=False,
                   num_devices=8)

    def din(name, shape):
        return nc.dram_tensor(name, shape, f32, kind="ExternalInput").ap()

    def dout(name, shape):
        return nc.dram_tensor(name, shape, f32, kind="ExternalOutput").ap()

    xt_d = din("xt", [2, 128, NCOLS])
    mb_d = din("mb", [2, 128, NCOLS])
    wih_d = din("wih", [128, 12 * 128])      # col block (k*6+m)
    whh_d = din("whh", [128, 12 * 128])
    biasv_d = din("biasv", [128, 6])         # bir0,bir1,bin0,bin1,bhn0,bhn1
    w1t_d = din("w1t", [128, 3 * 8 * 128])   # (s*8 + k*2 + m)
    w2t_d = din("w2t", [128, 3 * 4 * 128])   # (s*4 + k*2 + m)
    w3t_d = din("w3t", [128, 3 * 2 * W3PAD])  # (s*2 + k)
    extras_d = din("extras", [128, 3 * 80])  # per scale folded [m|m] x 40
    swew_d = din("swew", [40, 3 * 160])      # per s: sw(80) | ew(80)
    sevec_d = din("sevec", [40, 8])          # st s0..s2, en s0..s2, al, 0

    bounds_d = dout("bounds", [70, 2])
    hdump_d = dout("hdump", [128, 2 * IPAD])
    conf_d = dout("conf", [70, 1])
    cls_d = dout("cls", [70, 4])

    with TileContext(nc) as tc:
        with (
            tc.tile_pool(name="const", bufs=1) as constp,
            tc.tile_pool(name="xin", bufs=3) as xinp,
            tc.tile_pool(name="mbin", bufs=3) as mbinp,
            tc.tile_pool(name="gir", bufs=3) as girp,
            tc.tile_pool(name="state", bufs=1) as statep,
            tc.tile_pool(name="gates", bufs=2) as gatesp,
            tc.tile_pool(name="ps", bufs=2, space="PSUM") as psp,
            tc.tile_pool(name="psg", bufs=2, space="PSUM") as psgp,
            tc.tile_pool(name="mlp", bufs=1) as mlpp,
            tc.tile_pool(name="dram", bufs=1, space="DRAM") as dramp,
        ):
            # ---- constants ----
            whh_sb = constp.tile([128, 12 * 128], f32)
            nc.sync.dma_start(whh_sb[:], whh_d)
            wih_sb = constp.tile([128, 12 * 128], f32)
            nc.sync.dma_start(wih_sb[:], wih_d)
            biasv = constp.tile([128, 6], f32)
            nc.sync.dma_start(biasv[:], biasv_d)

            def whh_t(k, m):
                return whh_sb[:, (k * 6 + m) * 128:(k * 6 + m + 1) * 128]

            def wih_t(k, m):
                return wih_sb[:, (k * 6 + m) * 128:(k * 6 + m + 1) * 128]

            hfold = statep.tile([128, 2 * IPAD], f32)
            nc.vector.memset(hfold[:], 0.0)

            # ---- gi chunk machinery ----
            xt_tiles = {}
            mb_tiles = {}
            gi_tiles = {}

            def emit_chunk_dma(c):
                xt_sb = xinp.tile([128, 2 * CH], f32, tag="xt", name=f"xt_sb{c}")
                nc.sync.dma_start(xt_sb[:, 0:CH], xt_d[0, :, c * CH:(c + 1) * CH])
                nc.sync.dma_start(xt_sb[:, CH:2 * CH], xt_d[1, :, c * CH:(c + 1) * CH])
                mb_sb = mbinp.tile([128, 2 * CH], f32, tag="mb", name=f"mb_sb{c}")
                nc.sync.dma_start(mb_sb[:, 0:CH], mb_d[0, :, c * CH:(c + 1) * CH])
                nc.sync.dma_start(mb_sb[:, CH:2 * CH], mb_d[1, :, c * CH:(c + 1) * CH])
                xt_tiles[c] = xt_sb
                mb_tiles[c] = mb_sb
                gi_tiles[c] = girp.tile([128, 6 * CH], f32, tag="gi", name=f"gi_sb{c}")

            def emit_gi_piece(c, m):
                """One m-plane of chunk c's gi: 2 accumulating MMs + writeout."""
                xt_sb = xt_tiles[c]
                psg = psgp.tile([128, CH], f32, tag="psg", name=f"psg{c}_{m}")
                for k in range(2):
                    nc.tensor.matmul(psg[:], wih_t(k, m),
                                     xt_sb[:, k * CH:(k + 1) * CH],
                                     start=(k == 0), stop=(k == 1))
                gi_sb = gi_tiles[c]
                # strided view: step-major blocks of 6*IPAD cols, plane m at m*IPAD
                gi_view = gi_sb[:].rearrange("p (s c) -> p s c", c=6 * IPAD)
                out_ap = gi_view[:, :, m * IPAD:(m + 1) * IPAD]
                ps_view = psg[:].rearrange("p (s c) -> p s c", c=IPAD)
                if m in (2, 3):  # z planes: add maskbias (bias + 50*frozen)
                    mb_sb = mb_tiles[c]
                    mb_view = mb_sb[:, (m - 2) * CH:(m - 1) * CH].rearrange(
                        "p (s c) -> p s c", c=IPAD)
                    nc.vector.tensor_tensor(out_ap, ps_view, mb_view, op=Alu.add)
                else:
                    bj = {0: 0, 1: 1, 4: 2, 5: 3}[m]
                    nc.scalar.activation(out_ap, ps_view, Act.Identity,
                                         bias=biasv[:, bj:bj + 1])

            # ---- scan step ----
            def emit_scan_step(t):
                c = t // S
                lt = t % S
                gi_sb = gi_tiles[c]
                base = lt * 6 * IPAD
                ps_rz = psp.tile([128, 4 * IPAD], f32, tag="ps_rz")
                ps_n = psp.tile([128, 2 * IPAD], f32, tag="ps_n")
                for m in range(6):
                    tgt = (ps_rz[:, m * IPAD:(m + 1) * IPAD] if m < 4
                           else ps_n[:, (m - 4) * IPAD:(m - 3) * IPAD])
                    for k in range(2):
                        nc.tensor.matmul(tgt, whh_t(k, m),
                                         hfold[:, k * IPAD:(k + 1) * IPAD],
                                         start=(k == 0), stop=(k == 1))
                tmp_rz = gatesp.tile([128, 4 * IPAD], f32, tag="tmp_rz")
                nc.vector.tensor_tensor(tmp_rz[:], ps_rz[:],
                                        gi_sb[:, base:base + 4 * IPAD],
                                        op=Alu.add)
                rz_a = gatesp.tile([128, 4 * IPAD], f32, tag="rz_a")
                nc.scalar.activation(rz_a[:], tmp_rz[:], Act.Sigmoid)
                rhn = gatesp.tile([128, 2 * IPAD], f32, tag="rhn")
                for k in range(2):
                    nc.vector.scalar_tensor_tensor(
                        rhn[:, k * IPAD:(k + 1) * IPAD],
                        ps_n[:, k * IPAD:(k + 1) * IPAD],
                        biasv[:, 4 + k:5 + k],
                        rz_a[:, k * IPAD:(k + 1) * IPAD],
                        op0=Alu.add, op1=Alu.mult)
                npre = gatesp.tile([128, 2 * IPAD], f32, tag="npre")
                nc.vector.tensor_tensor(npre[:], rhn[:],
                                        gi_sb[:, base + 4 * IPAD:base + 6 * IPAD],
                                        op=Alu.add)
                n_a = gatesp.tile([128, 2 * IPAD], f32, tag="n_a")
                nc.scalar.activation(n_a[:], npre[:], Act.Tanh)
                dmn = gatesp.tile([128, 2 * IPAD], f32, tag="dmn")
                nc.vector.tensor_tensor(dmn[:], hfold[:], n_a[:], op=Alu.subtract)
                zd = gatesp.tile([128, 2 * IPAD], f32, tag="zd")
                nc.vector.tensor_tensor(zd[:], rz_a[:, 2 * IPAD:4 * IPAD], dmn[:],
                                        op=Alu.mult)
                nc.vector.tensor_tensor(hfold[:], n_a[:], zd[:], op=Alu.add)

            # ---- interleaved emission: prologue 2 chunks, then 1 gi piece/step
            for c in range(min(2, n_chunks)):
                emit_chunk_dma(c)
                for m in range(6):
                    emit_gi_piece(c, m)
            for t in range(T_pad):
                cs = t // S
                lt = t % S
                cg = cs + 2
                if cg < n_chunks:
                    if lt == 0:
                        emit_chunk_dma(cg)
                    if lt % 2 == 0 and lt // 2 < 6:
                        emit_gi_piece(cg, lt // 2)
                emit_scan_step(t)
                # release chunk tiles we no longer reference
                if lt == S - 1:
                    xt_tiles.pop(cs, None)
                    mb_tiles.pop(cs, None)
                    gi_tiles.pop(cs, None)

            # ---- AllGather final hiddens ----
            nc.sync.dma_start(hdump_d, hfold[:])
            ag_in = dramp.tile([128, 2 * IPAD], f32)
            ag_out = dramp.tile([8, 128, 2 * IPAD], f32)
            nc.sync.dma_start(ag_in[:], hfold[:])
            nc.gpsimd.collective_compute(
                "AllGather", mybir.AluOpType.bypass,
                replica_groups=[list(range(8))],
                ins=[ag_in.opt()], outs=[ag_out.opt()])

            # ---- MLP head (redundant on every core) ----
            w1_sb = mlpp.tile([128, 3 * 8 * 128], f32)
            nc.sync.dma_start(w1_sb[:], w1t_d)
            w2_sb = mlpp.tile([128, 3 * 4 * 128], f32)
            nc.sync.dma_start(w2_sb[:], w2t_d)
            w3_sb = mlpp.tile([128, 3 * 2 * W3PAD], f32)
            nc.sync.dma_start(w3_sb[:], w3t_d)
            extras_sb = mlpp.tile([128, 3 * 80], f32)
            nc.sync.dma_start(extras_sb[:], extras_d)
            swew_sb = mlpp.tile([40, 3 * 160], f32)
            nc.sync.dma_start(swew_sb[:], swew_d)
            sevec_sb = mlpp.tile([40, 8], f32)
            nc.sync.dma_start(sevec_sb[:], sevec_d)

            # assemble featT k-tiles [128, 70] from AllGather slots
            featk = []
            for k in range(4):
                fk = mlpp.tile([128, 70], f32, tag=f"featk{k}")
                featk.append(fk)
            for s in range(3):
                I = COUNTS[s]
                off = OFFS[s]
                fc, bc = 2 * s, 2 * s + 1
                for dcore, ks in ((fc, (0, 1)), (bc, (2, 3))):
                    for kk, k in enumerate(ks):
                        nc.sync.dma_start(
                            featk[k][:, off:off + I],
                            ag_out[dcore, :, kk * IPAD:kk * IPAD + I])

            bounds_sb = mlpp.tile([40, 2 * 3], f32, tag="bsb")  # per scale 2 cols
            conf_sb = mlpp.tile([40, 3], f32, tag="csb")
            cls_sb = mlpp.tile([40, 4 * 3], f32, tag="clsb")

            for s in range(3):
                I = COUNTS[s]
                b = BINS[s]
                W = 2 * b + 5
                off = OFFS[s]

                def w1_t(k, m):
                    j = s * 8 + k * 2 + m
                    return w1_sb[:, j * 128:(j + 1) * 128]

                def w2_t(k, m):
                    j = s * 4 + k * 2 + m
                    return w2_sb[:, j * 128:(j + 1) * 128]

                ps1 = psp.tile([128, 2 * IPAD], f32, tag="ps_rz", name=f"ps1_{s}")
                for m in range(2):
                    for k in range(4):
                        nc.tensor.matmul(ps1[:, m * I:(m + 1) * I], w1_t(k, m),
                                         featk[k][:, off:off + I],
                                         start=(k == 0), stop=(k == 3))
                h1 = mlpp.tile([128, 2 * IPAD], f32, tag="h1")
                nc.vector.tensor_tensor(h1[:, 0:2 * I], ps1[:, 0:2 * I],
                                        extras_sb[:, s * 80:s * 80 + 2 * I],
                                        op=Alu.add)
                nc.scalar.activation(h1[:, 0:2 * I], h1[:, 0:2 * I], Act.Relu)

                ps2 = psp.tile([128, 2 * IPAD], f32, tag="ps_n", name=f"ps2_{s}")
                for m in range(2):
                    for k in range(2):
                        nc.tensor.matmul(ps2[:, m * I:(m + 1) * I], w2_t(k, m),
                                         h1[:, k * I:(k + 1) * I],
                                         start=(k == 0), stop=(k == 1))
                h2 = mlpp.tile([128, 2 * IPAD], f32, tag="h2")
                nc.scalar.activation(h2[:, 0:2 * I], ps2[:, 0:2 * I], Act.Relu)

                ps3 = psgp.tile([40, W3PAD], f32, tag="psg", name=f"ps3_{s}")
                for k in range(2):
                    nc.tensor.matmul(
                        ps3[:I, 0:W], h2[:, k * I:(k + 1) * I],
                        w3_sb[:, (s * 2 + k) * W3PAD:(s * 2 + k) * W3PAD + W],
                        start=(k == 0), stop=(k == 1))

                # soft-argmax over start/end bins + clip
                for side in range(2):
                    seg = ps3[:I, side * b:(side + 1) * b]
                    nrmax = mlpp.tile([40, 1], f32, tag="nrmax")
                    nc.vector.tensor_reduce(nrmax[:I], seg,
                                            axis=mybir.AxisListType.X,
                                            op=Alu.max, negate=True)
                    esum = mlpp.tile([40, 1], f32, tag="esum")
                    etile = mlpp.tile([40, 80], f32, tag="etile")
                    nc.scalar.activation(etile[:I, 0:b], seg, Act.Exp,
                                         bias=nrmax[:I], accum_out=esum[:I])
                    wtile = mlpp.tile([40, 80], f32, tag="wtile")
                    wsum = mlpp.tile([40, 1], f32, tag="wsum")
                    wcol = swew_sb[:I, s * 160 + side * 80:s * 160 + side * 80 + b]
                    nc.vector.tensor_tensor(wtile[:I, 0:b], etile[:I, 0:b],
                                            wcol, op=Alu.mult)
                    nc.vector.tensor_reduce(wsum[:I], wtile[:I, 0:b],
                                            axis=mybir.AxisListType.X,
                                            op=Alu.add)
                    rec = mlpp.tile([40, 1], f32, tag="rec")
                    nc.vector.reciprocal(rec[:I], esum[:I])
                    so = mlpp.tile([40, 1], f32, tag="so")
                    nc.vector.tensor_tensor(so[:I], wsum[:I], rec[:I], op=Alu.mult)
                    anch = sevec_sb[:I, side * 3 + s:side * 3 + s + 1]
                    bsum = mlpp.tile([40, 1], f32, tag="bsum")
                    nc.vector.tensor_tensor(bsum[:I], so[:I], anch, op=Alu.add)
                    nc.vector.tensor_scalar(
                        bounds_sb[:I, s * 2 + side:s * 2 + side + 1], bsum[:I],
                        0.0, sevec_sb[:I, 6:7], op0=Alu.max, op1=Alu.min)

                nc.scalar.copy(conf_sb[:I, s:s + 1], ps3[:I, 2 * b:2 * b + 1])
                nc.scalar.copy(cls_sb[:I, s * 4:s * 4 + 4],
                               ps3[:I, 2 * b + 1:2 * b + 5])

            # ---- outputs ----
            for s in range(3):
                I = COUNTS[s]
                off = OFFS[s]
                nc.sync.dma_start(bounds_d[off:off + I, :],
                                  bounds_sb[:I, s * 2:s * 2 + 2])
                nc.sync.dma_start(conf_d[off:off + I, :], conf_sb[:I, s:s + 1])
                nc.sync.dma_start(cls_d[off:off + I, :],
                                  cls_sb[:I, s * 4:s * 4 + 4])

    nc.compile()
    return nc


def _build_inmaps(inputs, pre, T_pad):
    NCOLS = IPAD * T_pad
    ne = pre["ne"]
    eff = pre["eff"]
    counts = pre["counts"]
    members = pre["members"]
    orders = pre["orders"]
    al = pre["al"]
    anc = pre["anc"]
    abn = pre["abn"]

    gru_wih = np.asarray(inputs["gru_wih"], np.float32)
    gru_whh = np.asarray(inputs["gru_whh"], np.float32)
    gru_bih = np.asarray(inputs["gru_bih"], np.float32)
    gru_bhh = np.asarray(inputs["gru_bhh"], np.float32)
    w1 = np.asarray(inputs["w1"], np.float32)
    w2 = np.asarray(inputs["w2"], np.float32)

    # shared MLP constants
    w1t_a = np.zeros((128, 3 * 8 * 128), np.float32)
    w2t_a = np.zeros((128, 3 * 4 * 128), np.float32)
    w3t_a = np.zeros((128, 3 * 2 * W3PAD), np.float32)
    extras_a = np.zeros((128, 3 * 80), np.float32)
    swew_a = np.zeros((40, 3 * 160), np.float32)
    sevec_a = np.zeros((40, 8), np.float32)
    sevec_a[:, 6] = al
    w1qs, w2qs, w3qs = [], [], []
    for s in range(3):
        I = COUNTS[s]
        b = BINS[s]
        w1q = _quant(w1[s])
        w2q = _quant(w2[s])
        w3q = _quant(np.asarray(inputs[f"w3_s{s}"], np.float32))
        w1qs.append(w1q), w2qs.append(w2q), w3qs.append(w3q)
        w1T = w1q[:, 0:512].T.copy()          # [512, 256]
        for k in range(4):
            for m in range(2):
                j = s * 8 + k * 2 + m
                w1t_a[:, j * 128:(j + 1) * 128] = \
                    w1T[k * 128:(k + 1) * 128, m * 128:(m + 1) * 128]
        w2T = w2q.T.copy()
        for k in range(2):
            for m in range(2):
                j = s * 4 + k * 2 + m
                w2t_a[:, j * 128:(j + 1) * 128] = \
                    w2T[k * 128:(k + 1) * 128, m * 128:(m + 1) * 128]
        w3T = w3q.T.copy()                    # [256, W]
        W = 2 * b + 5
        for k in range(2):
            w3t_a[:, (s * 2 + k) * W3PAD:(s * 2 + k) * W3PAD + W] = \
                w3T[k * 128:(k + 1) * 128, :]
        order = pre["orders"][s]
        starts = anc[order, 0]
        ends = anc[order, 1]
        li = order - OFFS[s]
        extras = np.zeros((I, 7), np.float32)
        extras[:, 0] = abn[li]
        extras[:, 1] = (starts + ends) * 0.5 / al
        extras[:, 2] = (ends - starts) / al
        ec = (extras @ w1q[:, 512:519].T).astype(np.float32)  # [I, 256]
        ecT = ec.T                                            # [256, I]
        extras_a[:, s * 80:s * 80 + I] = ecT[0:128]
        extras_a[:, s * 80 + I:s * 80 + 2 * I] = ecT[128:256]
        swew_a[:I, s * 160:s * 160 + b] = np.asarray(inputs[f"sw_s{s}"],
                                                     np.float32)[None, :]
        swew_a[:I, s * 160 + 80:s * 160 + 80 + b] = np.asarray(
            inputs[f"ew_s{s}"], np.float32)[None, :]
        sevec_a[:I, s] = starts
        sevec_a[:I, 3 + s] = ends

    in_maps = []
    for core in range(8):
        s, d = CORE_GROUPS[core]
        I = COUNTS[s]
        order = orders[s]
        wih = gru_wih[s, d]
        whh = gru_whh[s, d]
        bih = gru_bih[s, d]
        bhh = gru_bhh[s, d]

        # packed X (transposed, k-planes): [2, 128, IPAD*T_pad]
        # column index = t*IPAD + j
        Xcols = np.zeros((T_pad, IPAD, D), np.float32)
        frozen = np.ones((T_pad, IPAD), np.float32)
        for j, i in enumerate(order):
            c = min(int(counts[i]), T_pad)
            idx = members[i] if d == 0 else members[i][::-1]
            if c:
                Xcols[:c, j, :] = ne[idx[:c]]
            frozen[:eff[i], j] = 0.0
        XT = Xcols.reshape(T_pad * IPAD, D).T.copy()     # [256, NCOLS]
        xt_a = np.stack([XT[0:128], XT[128:256]])        # [2,128,NCOLS]

        biz = (bih[H:2 * H] + bhh[H:2 * H]).astype(np.float32)
        mb_flat = 50.0 * frozen.reshape(1, NCOLS)
        mb_a = np.stack([biz[0:128, None] + mb_flat, biz[128:256, None] + mb_flat]
                        ).astype(np.float32)

        wihT = wih.T.copy()   # [256, 768]
        whhT = whh.T.copy()
        wih_a = np.zeros((128, 12 * 128), np.float32)
        whh_a = np.zeros((128, 12 * 128), np.float32)
        for k in range(2):
            for m in range(6):
                j = k * 6 + m
                wih_a[:, j * 128:(j + 1) * 128] = \
                    wihT[k * 128:(k + 1) * 128, m * 128:(m + 1) * 128]
                whh_a[:, j * 128:(j + 1) * 128] = \
                    whhT[k * 128:(k + 1) * 128, m * 128:(m + 1) * 128]

        bir = (bih[0:H] + bhh[0:H]).astype(np.float32)
        bin_ = bih[2 * H:3 * H].astype(np.float32)
        bhn = bhh[2 * H:3 * H].astype(np.float32)
        biasv_a = np.zeros((128, 6), np.float32)
        biasv_a[:, 0] = bir[0:128]
        biasv_a[:, 1] = bir[128:256]
        biasv_a[:, 2] = bin_[0:128]
        biasv_a[:, 3] = bin_[128:256]
        biasv_a[:, 4] = bhn[0:128]
        biasv_a[:, 5] = bhn[128:256]

        in_maps.append(dict(
            xt=xt_a, mb=mb_a, wih=wih_a, whh=whh_a, biasv=biasv_a,
            w1t=w1t_a, w2t=w2t_a, w3t=w3t_a, extras=extras_a,
            swew=swew_a, sevec=sevec_a))
    return in_maps


def kernel(**inputs):
    from concourse import bass_utils

    pre = _preprocess(inputs)
    T_pad = max(2 * S, int(math.ceil(pre["T"] / S)) * S)
    if T_pad not in _prog_cache:
        _prog_cache[T_pad] = _build_program(T_pad)
    nc = _prog_cache[T_pad]
    in_maps = _build_inmaps(inputs, pre, T_pad)
    kwargs = {}
    if PROFILE:
        try:
            import ntff_hook  # noqa: F401  (registers the axon NTFF hook)
        except Exception:
            pass
        kwargs = dict(trace=True, trace_cores=TRACE_CORES)
    res = bass_utils.run_bass_kernel_spmd(nc, in_maps,
                                          core_ids=list(range(8)), **kwargs)
    global LAST_RESULT
    LAST_RESULT = res
    r0 = res.results[0]

    bounds = np.zeros((70, 2), np.float32)
    conf = np.zeros((70,), np.float32)
    cls = np.zeros((70, 4), np.float32)
    for s in range(3):
        order = pre["orders"][s]
        off = OFFS[s]
        I = COUNTS[s]
        bounds[order] = r0["bounds"][off:off + I]
        conf[order] = r0["conf"][off:off + I, 0]
        cls[order] = r0["cls"][off:off + I]
    return bounds, conf, cls


# revision 8
# speedup vs baseline: 1.8912x; 1.8912x over previous
"""Trainium2 Bass kernel for nn_Interval_Refine (ragged bidirectional GRU +
quantized MLP interval refinement).

Sharding: 6 of 8 cores each own one (scale, direction) GRU group — the
sequential scan is the bottleneck, and groups are independent. Host packs
each group's member-node sequences (sorted by length, rectangular I=40
columns) ; frozen steps (t >= eff) get +50 on the z-gate pre-activation so
z == 1 exactly in fp32 and h carries unchanged. Per scan step the core does
12 accumulating matmuls (h @ whh.T in transposed/folded layout) plus fused
gate math on ACT/DVE; the input-side gi = x @ wih.T matmuls are interleaved
one-per-step into the PE stream so they run inside the gate-latency gaps.
After the scan an AllGather shares final hiddens; every core redundantly
computes the (int8-fake-quant) MLP head + soft-argmax boundary refinement.
"""

import math

import ml_dtypes
import numpy as np

COUNTS = [15, 40, 15]
OFFS = [0, 15, 55, 70]
BINS = [80, 60, 80]
MAX_T = 512
IPAD = 40
D = 256
H = 256
S = 12  # scan steps per gi chunk
W3PAD = 168
# core -> (scale, dir); cores 6,7 duplicate scale 1 (their slots are ignored)
CORE_GROUPS = [(0, 0), (0, 1), (1, 0), (1, 1), (2, 0), (2, 1), (1, 0), (1, 1)]

_prog_cache = {}
PROFILE = False          # set True to capture an NTFF profile on the next run
TRACE_CORES = None       # e.g. [4] to profile the scale-2 fwd core
LAST_RESULT = None       # BassKernelResults of the most recent run


def _quant(w):
    w = np.asarray(w, np.float32)
    s = np.maximum((np.abs(w).max() / np.float32(127.0)).astype(np.float32),
                   np.float32(1e-8))
    q = np.clip(np.round(w / s), -128, 127).astype(np.float32) * s
    return q.astype(np.float32)


def _softmax(x, axis=-1):
    m = x.max(axis=axis, keepdims=True)
    e = np.exp(x - m)
    return e / e.sum(axis=axis, keepdims=True)


def _preprocess(inputs):
    ne = np.asarray(inputs["node_embeddings"], np.float32)
    tp = np.asarray(inputs["time_positions"], np.float32)[:, 0]
    al = np.float32(np.asarray(inputs["audio_len"], np.float32)[0])
    anc = np.asarray(inputs["anchor_intervals"], np.float32)
    tpos = tp * al

    members, counts = [], []
    for i in range(70):
        m = np.nonzero((tpos >= anc[i, 0]) & (tpos <= anc[i, 1]))[0][:MAX_T]
        members.append(m)
        counts.append(len(m))
    counts = np.array(counts)
    eff = np.clip(counts, 1, MAX_T)
    orders = [OFFS[s] + np.argsort(-eff[OFFS[s]:OFFS[s + 1]], kind="stable")
              for s in range(3)]
    T = int(eff.max())
    abn = _softmax(np.asarray(inputs["node_pred"], np.float32), axis=-1)[:, 0]
    return dict(ne=ne, members=members, counts=counts, eff=eff, orders=orders,
                T=T, tpos=tpos, al=al, anc=anc, abn=abn)


def _build_program(T_pad):
    import concourse.bacc as bacc
    import concourse.mybir as mybir
    from concourse.tile import TileContext

    f32 = mybir.dt.float32
    Alu = mybir.AluOpType
    Act = mybir.ActivationFunctionType

    NCOLS = IPAD * T_pad
    CH = IPAD * S  # 480 columns per gi chunk
    n_chunks = T_pad // S

    nc = bacc.Bacc("TRN2", target_bir_lowering=False, debug=False,
                   num_devices=8)

    bf16 = mybir.dt.bfloat16

    def din(name, shape, dt=f32):
        return nc.dram_tensor(name, shape, dt, kind="ExternalInput").ap()

    def dout(name, shape):
        return nc.dram_tensor(name, shape, f32, kind="ExternalOutput").ap()

    xt_d = din("xt", [2, 128, NCOLS], bf16)
    mb_d = din("mb", [2, 128, NCOLS])
    wih_d = din("wih", [128, 12 * 128], bf16)  # col block (k*6+m)
    whh_d = din("whh", [128, 12 * 128], bf16)
    biasv_d = din("biasv", [128, 6])         # bir0,bir1,bin0,bin1,bhn0,bhn1
    w1t_d = din("w1t", [128, 3 * 8 * 128])   # (s*8 + k*2 + m)
    w2t_d = din("w2t", [128, 3 * 4 * 128])   # (s*4 + k*2 + m)
    w3t_d = din("w3t", [128, 3 * 2 * W3PAD])  # (s*2 + k)
    extras_d = din("extras", [128, 3 * 80])  # per scale folded [m|m] x 40
    swew_d = din("swew", [40, 3 * 160])      # per s: sw(80) | ew(80)
    sevec_d = din("sevec", [40, 8])          # st s0..s2, en s0..s2, al, 0

    bounds_d = dout("bounds", [70, 2])
    hdump_d = dout("hdump", [128, 2 * IPAD])
    conf_d = dout("conf", [70, 1])
    cls_d = dout("cls", [70, 4])

    with TileContext(nc) as tc:
        with (
            tc.tile_pool(name="const", bufs=1) as constp,
            tc.tile_pool(name="xin", bufs=3) as xinp,
            tc.tile_pool(name="mbin", bufs=3) as mbinp,
            tc.tile_pool(name="gir", bufs=3) as girp,
            tc.tile_pool(name="state", bufs=1) as statep,
            tc.tile_pool(name="gates", bufs=2) as gatesp,
            tc.tile_pool(name="ps", bufs=2, space="PSUM") as psp,
            tc.tile_pool(name="psg", bufs=2, space="PSUM") as psgp,
            tc.tile_pool(name="mlp", bufs=1) as mlpp,
            tc.tile_pool(name="dram", bufs=1, space="DRAM") as dramp,
        ):
            # ---- constants ----
            whh_sb = constp.tile([128, 12 * 128], bf16)
            nc.sync.dma_start(whh_sb[:], whh_d)
            wih_sb = constp.tile([128, 12 * 128], bf16)
            nc.sync.dma_start(wih_sb[:], wih_d)
            biasv = constp.tile([128, 6], f32)
            nc.sync.dma_start(biasv[:], biasv_d)

            def whh_t(k, m):
                return whh_sb[:, (k * 6 + m) * 128:(k * 6 + m + 1) * 128]

            def wih_t(k, m):
                return wih_sb[:, (k * 6 + m) * 128:(k * 6 + m + 1) * 128]

            hfold = statep.tile([128, 2 * IPAD], f32)
            nc.vector.memset(hfold[:], 0.0)
            hbf = statep.tile([128, 2 * IPAD], bf16)
            nc.vector.memset(hbf[:], 0.0)

            # ---- gi chunk machinery ----
            xt_tiles = {}
            mb_tiles = {}
            gi_tiles = {}

            def emit_chunk_dma(c):
                xt_sb = xinp.tile([128, 2 * CH], bf16, tag="xt", name=f"xt_sb{c}")
                nc.sync.dma_start(xt_sb[:, 0:CH], xt_d[0, :, c * CH:(c + 1) * CH])
                nc.sync.dma_start(xt_sb[:, CH:2 * CH], xt_d[1, :, c * CH:(c + 1) * CH])
                mb_sb = mbinp.tile([128, 2 * CH], f32, tag="mb", name=f"mb_sb{c}")
                nc.sync.dma_start(mb_sb[:, 0:CH], mb_d[0, :, c * CH:(c + 1) * CH])
                nc.sync.dma_start(mb_sb[:, CH:2 * CH], mb_d[1, :, c * CH:(c + 1) * CH])
                xt_tiles[c] = xt_sb
                mb_tiles[c] = mb_sb
                gi_tiles[c] = girp.tile([128, 6 * CH], f32, tag="gi", name=f"gi_sb{c}")

            def emit_gi_piece(c, m):
                """One m-plane of chunk c's gi: 2 accumulating MMs + writeout."""
                xt_sb = xt_tiles[c]
                psg = psgp.tile([128, CH], f32, tag="psg", name=f"psg{c}_{m}")
                for k in range(2):
                    nc.tensor.matmul(psg[:], wih_t(k, m),
                                     xt_sb[:, k * CH:(k + 1) * CH],
                                     start=(k == 0), stop=(k == 1))
                gi_sb = gi_tiles[c]
                # strided view: step-major blocks of 6*IPAD cols, plane m at m*IPAD
                gi_view = gi_sb[:].rearrange("p (s c) -> p s c", c=6 * IPAD)
                out_ap = gi_view[:, :, m * IPAD:(m + 1) * IPAD]
                ps_view = psg[:].rearrange("p (s c) -> p s c", c=IPAD)
                if m in (2, 3):  # z planes: add maskbias (bias + 50*frozen)
                    mb_sb = mb_tiles[c]
                    mb_view = mb_sb[:, (m - 2) * CH:(m - 1) * CH].rearrange(
                        "p (s c) -> p s c", c=IPAD)
                    nc.vector.tensor_tensor(out_ap, ps_view, mb_view, op=Alu.add)
                else:
                    bj = {0: 0, 1: 1, 4: 2, 5: 3}[m]
                    nc.scalar.activation(out_ap, ps_view, Act.Identity,
                                         bias=biasv[:, bj:bj + 1])

            # ---- scan step ----
            def emit_scan_step(t):
                c = t // S
                lt = t % S
                gi_sb = gi_tiles[c]
                base = lt * 6 * IPAD
                ps_rz = psp.tile([128, 4 * IPAD], f32, tag="ps_rz")
                ps_n = psp.tile([128, 2 * IPAD], f32, tag="ps_n")
                for m in range(6):
                    tgt = (ps_rz[:, m * IPAD:(m + 1) * IPAD] if m < 4
                           else ps_n[:, (m - 4) * IPAD:(m - 3) * IPAD])
                    for k in range(2):
                        nc.tensor.matmul(tgt, whh_t(k, m),
                                         hbf[:, k * IPAD:(k + 1) * IPAD],
                                         start=(k == 0), stop=(k == 1))
                tmp_rz = gatesp.tile([128, 4 * IPAD], f32, tag="tmp_rz")
                nc.vector.tensor_tensor(tmp_rz[:], ps_rz[:],
                                        gi_sb[:, base:base + 4 * IPAD],
                                        op=Alu.add)
                rz_a = gatesp.tile([128, 4 * IPAD], f32, tag="rz_a")
                nc.scalar.activation(rz_a[:], tmp_rz[:], Act.Sigmoid)
                rhn = gatesp.tile([128, 2 * IPAD], f32, tag="rhn")
                for k in range(2):
                    nc.vector.scalar_tensor_tensor(
                        rhn[:, k * IPAD:(k + 1) * IPAD],
                        ps_n[:, k * IPAD:(k + 1) * IPAD],
                        biasv[:, 4 + k:5 + k],
                        rz_a[:, k * IPAD:(k + 1) * IPAD],
                        op0=Alu.add, op1=Alu.mult)
                npre = gatesp.tile([128, 2 * IPAD], f32, tag="npre")
                nc.vector.tensor_tensor(npre[:], rhn[:],
                                        gi_sb[:, base + 4 * IPAD:base + 6 * IPAD],
                                        op=Alu.add)
                n_a = gatesp.tile([128, 2 * IPAD], f32, tag="n_a")
                nc.scalar.activation(n_a[:], npre[:], Act.Tanh)
                dmn = gatesp.tile([128, 2 * IPAD], f32, tag="dmn")
                nc.vector.tensor_tensor(dmn[:], hfold[:], n_a[:], op=Alu.subtract)
                zd = gatesp.tile([128, 2 * IPAD], f32, tag="zd")
                nc.vector.tensor_tensor(zd[:], rz_a[:, 2 * IPAD:4 * IPAD], dmn[:],
                                        op=Alu.mult)
                nc.vector.tensor_tensor(hfold[:], n_a[:], zd[:], op=Alu.add)
                nc.vector.tensor_copy(out=hbf[:], in_=hfold[:])

            # ---- interleaved emission: prologue 2 chunks, then 1 gi piece/step
            for c in range(min(2, n_chunks)):
                emit_chunk_dma(c)
                for m in range(6):
                    emit_gi_piece(c, m)
            for t in range(T_pad):
                cs = t // S
                lt = t % S
                cg = cs + 2
                if cg < n_chunks:
                    if lt == 0:
                        emit_chunk_dma(cg)
                    if lt % 2 == 0 and lt // 2 < 6:
                        emit_gi_piece(cg, lt // 2)
                emit_scan_step(t)
                # release chunk tiles we no longer reference
                if lt == S - 1:
                    xt_tiles.pop(cs, None)
                    mb_tiles.pop(cs, None)
                    gi_tiles.pop(cs, None)

            # ---- AllGather final hiddens ----
            nc.sync.dma_start(hdump_d, hfold[:])
            ag_in = dramp.tile([128, 2 * IPAD], f32)
            ag_out = dramp.tile([8, 128, 2 * IPAD], f32)
            nc.sync.dma_start(ag_in[:], hfold[:])
            nc.gpsimd.collective_compute(
                "AllGather", mybir.AluOpType.bypass,
                replica_groups=[list(range(8))],
                ins=[ag_in.opt()], outs=[ag_out.opt()])

            # ---- MLP head (redundant on every core) ----
            w1_sb = mlpp.tile([128, 3 * 8 * 128], f32)
            nc.sync.dma_start(w1_sb[:], w1t_d)
            w2_sb = mlpp.tile([128, 3 * 4 * 128], f32)
            nc.sync.dma_start(w2_sb[:], w2t_d)
            w3_sb = mlpp.tile([128, 3 * 2 * W3PAD], f32)
            nc.sync.dma_start(w3_sb[:], w3t_d)
            extras_sb = mlpp.tile([128, 3 * 80], f32)
            nc.sync.dma_start(extras_sb[:], extras_d)
            swew_sb = mlpp.tile([40, 3 * 160], f32)
            nc.sync.dma_start(swew_sb[:], swew_d)
            sevec_sb = mlpp.tile([40, 8], f32)
            nc.sync.dma_start(sevec_sb[:], sevec_d)

            # assemble featT k-tiles [128, 70] from AllGather slots
            featk = []
            for k in range(4):
                fk = mlpp.tile([128, 70], f32, tag=f"featk{k}")
                featk.append(fk)
            for s in range(3):
                I = COUNTS[s]
                off = OFFS[s]
                fc, bc = 2 * s, 2 * s + 1
                for dcore, ks in ((fc, (0, 1)), (bc, (2, 3))):
                    for kk, k in enumerate(ks):
                        nc.sync.dma_start(
                            featk[k][:, off:off + I],
                            ag_out[dcore, :, kk * IPAD:kk * IPAD + I])

            bounds_sb = mlpp.tile([40, 2 * 3], f32, tag="bsb")  # per scale 2 cols
            conf_sb = mlpp.tile([40, 3], f32, tag="csb")
            cls_sb = mlpp.tile([40, 4 * 3], f32, tag="clsb")

            for s in range(3):
                I = COUNTS[s]
                b = BINS[s]
                W = 2 * b + 5
                off = OFFS[s]

                def w1_t(k, m):
                    j = s * 8 + k * 2 + m
                    return w1_sb[:, j * 128:(j + 1) * 128]

                def w2_t(k, m):
                    j = s * 4 + k * 2 + m
                    return w2_sb[:, j * 128:(j + 1) * 128]

                ps1 = psp.tile([128, 2 * IPAD], f32, tag="ps_rz", name=f"ps1_{s}")
                for m in range(2):
                    for k in range(4):
                        nc.tensor.matmul(ps1[:, m * I:(m + 1) * I], w1_t(k, m),
                                         featk[k][:, off:off + I],
                                         start=(k == 0), stop=(k == 3))
                h1 = mlpp.tile([128, 2 * IPAD], f32, tag="h1")
                nc.vector.tensor_tensor(h1[:, 0:2 * I], ps1[:, 0:2 * I],
                                        extras_sb[:, s * 80:s * 80 + 2 * I],
                                        op=Alu.add)
                nc.scalar.activation(h1[:, 0:2 * I], h1[:, 0:2 * I], Act.Relu)

                ps2 = psp.tile([128, 2 * IPAD], f32, tag="ps_n", name=f"ps2_{s}")
                for m in range(2):
                    for k in range(2):
                        nc.tensor.matmul(ps2[:, m * I:(m + 1) * I], w2_t(k, m),
                                         h1[:, k * I:(k + 1) * I],
                                         start=(k == 0), stop=(k == 1))
                h2 = mlpp.tile([128, 2 * IPAD], f32, tag="h2")
                nc.scalar.activation(h2[:, 0:2 * I], ps2[:, 0:2 * I], Act.Relu)

                ps3 = psgp.tile([40, W3PAD], f32, tag="psg", name=f"ps3_{s}")
                for k in range(2):
                    nc.tensor.matmul(
                        ps3[:I, 0:W], h2[:, k * I:(k + 1) * I],
                        w3_sb[:, (s * 2 + k) * W3PAD:(s * 2 + k) * W3PAD + W],
                        start=(k == 0), stop=(k == 1))

                # soft-argmax over start/end bins + clip
                for side in range(2):
                    seg = ps3[:I, side * b:(side + 1) * b]
                    nrmax = mlpp.tile([40, 1], f32, tag="nrmax")
                    nc.vector.tensor_reduce(nrmax[:I], seg,
                                            axis=mybir.AxisListType.X,
                                            op=Alu.max, negate=True)
                    esum = mlpp.tile([40, 1], f32, tag="esum")
                    etile = mlpp.tile([40, 80], f32, tag="etile")
                    nc.scalar.activation(etile[:I, 0:b], seg, Act.Exp,
                                         bias=nrmax[:I], accum_out=esum[:I])
                    wtile = mlpp.tile([40, 80], f32, tag="wtile")
                    wsum = mlpp.tile([40, 1], f32, tag="wsum")
                    wcol = swew_sb[:I, s * 160 + side * 80:s * 160 + side * 80 + b]
                    nc.vector.tensor_tensor(wtile[:I, 0:b], etile[:I, 0:b],
                                            wcol, op=Alu.mult)
                    nc.vector.tensor_reduce(wsum[:I], wtile[:I, 0:b],
                                            axis=mybir.AxisListType.X,
                                            op=Alu.add)
                    rec = mlpp.tile([40, 1], f32, tag="rec")
                    nc.vector.reciprocal(rec[:I], esum[:I])
                    so = mlpp.tile([40, 1], f32, tag="so")
                    nc.vector.tensor_tensor(so[:I], wsum[:I], rec[:I], op=Alu.mult)
                    anch = sevec_sb[:I, side * 3 + s:side * 3 + s + 1]
                    bsum = mlpp.tile([40, 1], f32, tag="bsum")
                    nc.vector.tensor_tensor(bsum[:I], so[:I], anch, op=Alu.add)
                    nc.vector.tensor_scalar(
                        bounds_sb[:I, s * 2 + side:s * 2 + side + 1], bsum[:I],
                        0.0, sevec_sb[:I, 6:7], op0=Alu.max, op1=Alu.min)

                nc.scalar.copy(conf_sb[:I, s:s + 1], ps3[:I, 2 * b:2 * b + 1])
                nc.scalar.copy(cls_sb[:I, s * 4:s * 4 + 4],
                               ps3[:I, 2 * b + 1:2 * b + 5])

            # ---- outputs ----
            for s in range(3):
                I = COUNTS[s]
                off = OFFS[s]
                nc.sync.dma_start(bounds_d[off:off + I, :],
                                  bounds_sb[:I, s * 2:s * 2 + 2])
                nc.sync.dma_start(conf_d[off:off + I, :], conf_sb[:I, s:s + 1])
                nc.sync.dma_start(cls_d[off:off + I, :],
                                  cls_sb[:I, s * 4:s * 4 + 4])

    nc.compile()
    return nc


def _build_inmaps(inputs, pre, T_pad):
    NCOLS = IPAD * T_pad
    ne = pre["ne"]
    eff = pre["eff"]
    counts = pre["counts"]
    members = pre["members"]
    orders = pre["orders"]
    al = pre["al"]
    anc = pre["anc"]
    abn = pre["abn"]

    gru_wih = np.asarray(inputs["gru_wih"], np.float32)
    gru_whh = np.asarray(inputs["gru_whh"], np.float32)
    gru_bih = np.asarray(inputs["gru_bih"], np.float32)
    gru_bhh = np.asarray(inputs["gru_bhh"], np.float32)
    w1 = np.asarray(inputs["w1"], np.float32)
    w2 = np.asarray(inputs["w2"], np.float32)

    # shared MLP constants
    w1t_a = np.zeros((128, 3 * 8 * 128), np.float32)
    w2t_a = np.zeros((128, 3 * 4 * 128), np.float32)
    w3t_a = np.zeros((128, 3 * 2 * W3PAD), np.float32)
    extras_a = np.zeros((128, 3 * 80), np.float32)
    swew_a = np.zeros((40, 3 * 160), np.float32)
    sevec_a = np.zeros((40, 8), np.float32)
    sevec_a[:, 6] = al
    w1qs, w2qs, w3qs = [], [], []
    for s in range(3):
        I = COUNTS[s]
        b = BINS[s]
        w1q = _quant(w1[s])
        w2q = _quant(w2[s])
        w3q = _quant(np.asarray(inputs[f"w3_s{s}"], np.float32))
        w1qs.append(w1q), w2qs.append(w2q), w3qs.append(w3q)
        w1T = w1q[:, 0:512].T.copy()          # [512, 256]
        for k in range(4):
            for m in range(2):
                j = s * 8 + k * 2 + m
                w1t_a[:, j * 128:(j + 1) * 128] = \
                    w1T[k * 128:(k + 1) * 128, m * 128:(m + 1) * 128]
        w2T = w2q.T.copy()
        for k in range(2):
            for m in range(2):
                j = s * 4 + k * 2 + m
                w2t_a[:, j * 128:(j + 1) * 128] = \
                    w2T[k * 128:(k + 1) * 128, m * 128:(m + 1) * 128]
        w3T = w3q.T.copy()                    # [256, W]
        W = 2 * b + 5
        for k in range(2):
            w3t_a[:, (s * 2 + k) * W3PAD:(s * 2 + k) * W3PAD + W] = \
                w3T[k * 128:(k + 1) * 128, :]
        order = pre["orders"][s]
        starts = anc[order, 0]
        ends = anc[order, 1]
        li = order - OFFS[s]
        extras = np.zeros((I, 7), np.float32)
        extras[:, 0] = abn[li]
        extras[:, 1] = (starts + ends) * 0.5 / al
        extras[:, 2] = (ends - starts) / al
        ec = (extras @ w1q[:, 512:519].T).astype(np.float32)  # [I, 256]
        ecT = ec.T                                            # [256, I]
        extras_a[:, s * 80:s * 80 + I] = ecT[0:128]
        extras_a[:, s * 80 + I:s * 80 + 2 * I] = ecT[128:256]
        swew_a[:I, s * 160:s * 160 + b] = np.asarray(inputs[f"sw_s{s}"],
                                                     np.float32)[None, :]
        swew_a[:I, s * 160 + 80:s * 160 + 80 + b] = np.asarray(
            inputs[f"ew_s{s}"], np.float32)[None, :]
        sevec_a[:I, s] = starts
        sevec_a[:I, 3 + s] = ends

    in_maps = []
    for core in range(8):
        s, d = CORE_GROUPS[core]
        I = COUNTS[s]
        order = orders[s]
        wih = gru_wih[s, d]
        whh = gru_whh[s, d]
        bih = gru_bih[s, d]
        bhh = gru_bhh[s, d]

        # packed X (transposed, k-planes): [2, 128, IPAD*T_pad]
        # column index = t*IPAD + j
        Xcols = np.zeros((T_pad, IPAD, D), np.float32)
        frozen = np.ones((T_pad, IPAD), np.float32)
        for j, i in enumerate(order):
            c = min(int(counts[i]), T_pad)
            idx = members[i] if d == 0 else members[i][::-1]
            if c:
                Xcols[:c, j, :] = ne[idx[:c]]
            frozen[:eff[i], j] = 0.0
        XT = Xcols.reshape(T_pad * IPAD, D).T.copy()     # [256, NCOLS]
        xt_a = np.stack([XT[0:128], XT[128:256]])        # [2,128,NCOLS]

        biz = (bih[H:2 * H] + bhh[H:2 * H]).astype(np.float32)
        mb_flat = 50.0 * frozen.reshape(1, NCOLS)
        mb_a = np.stack([biz[0:128, None] + mb_flat, biz[128:256, None] + mb_flat]
                        ).astype(np.float32)

        wihT = wih.T.copy()   # [256, 768]
        whhT = whh.T.copy()
        wih_a = np.zeros((128, 12 * 128), np.float32)
        whh_a = np.zeros((128, 12 * 128), np.float32)
        for k in range(2):
            for m in range(6):
                j = k * 6 + m
                wih_a[:, j * 128:(j + 1) * 128] = \
                    wihT[k * 128:(k + 1) * 128, m * 128:(m + 1) * 128]
                whh_a[:, j * 128:(j + 1) * 128] = \
                    whhT[k * 128:(k + 1) * 128, m * 128:(m + 1) * 128]

        bir = (bih[0:H] + bhh[0:H]).astype(np.float32)
        bin_ = bih[2 * H:3 * H].astype(np.float32)
        bhn = bhh[2 * H:3 * H].astype(np.float32)
        biasv_a = np.zeros((128, 6), np.float32)
        biasv_a[:, 0] = bir[0:128]
        biasv_a[:, 1] = bir[128:256]
        biasv_a[:, 2] = bin_[0:128]
        biasv_a[:, 3] = bin_[128:256]
        biasv_a[:, 4] = bhn[0:128]
        biasv_a[:, 5] = bhn[128:256]

        in_maps.append(dict(
            xt=xt_a.astype(ml_dtypes.bfloat16), mb=mb_a,
            wih=wih_a.astype(ml_dtypes.bfloat16),
            whh=whh_a.astype(ml_dtypes.bfloat16), biasv=biasv_a,
            w1t=w1t_a, w2t=w2t_a, w3t=w3t_a, extras=extras_a,
            swew=swew_a, sevec=sevec_a))
    return in_maps


def kernel(**inputs):
    from concourse import bass_utils

    pre = _preprocess(inputs)
    T_pad = max(2 * S, int(math.ceil(pre["T"] / S)) * S)
    if T_pad not in _prog_cache:
        _prog_cache[T_pad] = _build_program(T_pad)
    nc = _prog_cache[T_pad]
    in_maps = _build_inmaps(inputs, pre, T_pad)
    kwargs = {}
    if PROFILE:
        try:
            import ntff_hook  # noqa: F401  (registers the axon NTFF hook)
        except Exception:
            pass
        kwargs = dict(trace=True, trace_cores=TRACE_CORES)
    res = bass_utils.run_bass_kernel_spmd(nc, in_maps,
                                          core_ids=list(range(8)), **kwargs)
    global LAST_RESULT
    LAST_RESULT = res
    r0 = res.results[0]

    bounds = np.zeros((70, 2), np.float32)
    conf = np.zeros((70,), np.float32)
    cls = np.zeros((70, 4), np.float32)
    for s in range(3):
        order = pre["orders"][s]
        off = OFFS[s]
        I = COUNTS[s]
        bounds[order] = r0["bounds"][off:off + I]
        conf[order] = r0["conf"][off:off + I, 0]
        cls[order] = r0["cls"][off:off + I]
    return bounds, conf, cls
